# revision 14
# baseline (speedup 1.0000x reference)
"""Trainium2 Bass kernel for nn_ConvolutionalSelfAttention.

Mathematical simplification of the reference:
    v[b,t,o]  = sum_c x[b,t,c] W_attn[o,c]
    s[b,t]    = sum_o v[b,t,o] = sum_c x[b,t,c] * wa[c],   wa = colsum(W_attn)
    y[b,t]    = (s[b,t] + s[b,t-1] + s[b,t-2]) / 3        (zero-padded, causal)
    out[b,t,o]= y[b,t] * wp[o],                            wp = rowsum(W_proj)

Sharding (8 cores): core k owns 2048 consecutive tokens of batch k//2
(half k%2).  The tiny reduced weight vectors wa/3 and wp are computed on
the host during input prep (like the halo) and broadcast as needed;
there is no device-side collective.

Schedule (v10):
  - x pre-cast to bf16 on the host; out written as bf16 and upcast in
    assemble().  Device HBM traffic per core: 8 MB in + 8 MB out + ~1 MB
    weights (~46 us roofline at 358 GB/s per core).
  - Token layout: chunk c covers 128*CH_c tokens starting at base_c;
    token t = base_c + 128h + p sits at partition p, column h.  Chunk
    sizes ramp [2,3,3,4,4] so the pipeline fills early.
  - Engine split, balanced against the measured ~67 us copy-only floor
    (v8/v10 were DVE-bound at ~50 us busy).  Per token column: a product
    pass (x*wa), a free-axis reduce, and an outer-product pass
    (y[t]*wp).  These are spread so no engine exceeds ~32 us:
      DVE:    all product passes (bf16 2x); reduce fused via
              scalar_tensor_tensor accum_out for 4/16 columns; outer for
              5/16 columns; tiny copies.
      ACT:    reduce via activation-Copy accum_out for 12/16 columns;
              outer for 2/16; output DMA issue.
      GPSIMD: outer for 9/16 columns (tensor_scalar_mul, per-partition
              scalar); weight/const DMA descriptors.
      PE:     y via 2 matmuls/chunk: banded stationary W3 (k in
              {m-2,m-1,m}) for same-column taps + carry stationary CARS
              against the shifted column tile (prev column / prev chunk /
              host-prepared halo column hs).
  - Emission is software-pipelined with a 1-chunk skew (s-stage of chunk
    c emitted before y/outer stage of chunk c-1) so ACT's in-order queue
    always has next-chunk reduces available while y_c is in flight.
  - Queues: SP HWDGE = x chunks; gpsimd = weight vectors/constants;
    ACT HWDGE = output chunks.  All DMA payloads bf16 except tiny consts.
  - Rejected alternatives (measured): tensor_tensor_reduce crashes on HW
    (passes CoreSim); gpsimd scalar_tensor_tensor is rejected by
    neuronxcc; DMA-transpose + PE-matvec for s shatters into 256B
    descriptors and is ~2x slower end-to-end (v9: 157 us).
"""

import numpy as np
from contextlib import ExitStack

B, T, C = 4, 4096, 2048
N_CORES = 8
T_LOC = (B * T) // N_CORES      # 2048 tokens per core
P = 128
HC = C // 2
CHS = [2, 3, 3, 4, 4]           # token columns per chunk (x128 tokens)
NCH = len(CHS)
CHMAX = max(CHS)
assert sum(CHS) * P == T_LOC

_BUILT = {}


def _band_consts():
    # lhsT convention: out[m] = sum_k M[k, m] * v[k]
    w3 = np.zeros((P, P), np.float32)
    for m in range(P):
        for k in range(max(0, m - 2), m + 1):
            w3[k, m] = 1.0
    cars = np.zeros((P, P), np.float32)
    cars[P - 1, 0] = 1.0   # t-1 for partition 0
    cars[P - 2, 0] = 1.0   # t-2 for partition 0
    cars[P - 1, 1] = 1.0   # t-2 for partition 1
    return w3, cars


def _build_nc():
    import concourse.tile as tile
    from concourse import bacc, mybir

    f32 = mybir.dt.float32
    bf16 = mybir.dt.bfloat16
    AF = mybir.ActivationFunctionType
    ALU = mybir.AluOpType

    nc = bacc.Bacc("TRN2", target_bir_lowering=False, debug=False,
                   num_devices=N_CORES)

    x_shard = nc.dram_tensor("x_shard", [T_LOC, C], bf16, kind="ExternalInput")
    wa_bc_d = nc.dram_tensor("wa_bc", [P, C], bf16, kind="ExternalInput")
    wp_bc_d = nc.dram_tensor("wp_bc", [P, C], bf16, kind="ExternalInput")
    hs_d = nc.dram_tensor("hs", [P, 1], f32, kind="ExternalInput")
    w3_d = nc.dram_tensor("w3", [P, P], f32, kind="ExternalInput")
    cars_d = nc.dram_tensor("cars", [P, P], f32, kind="ExternalInput")
    out = nc.dram_tensor("out", [T_LOC, C], bf16, kind="ExternalOutput")

    with tile.TileContext(nc) as tc, ExitStack() as ctx:
        cpool = ctx.enter_context(tc.tile_pool(name="const", bufs=1))
        xpool = ctx.enter_context(tc.tile_pool(name="x", bufs=NCH))
        opool = ctx.enter_context(tc.tile_pool(name="o", bufs=2))
        spool = ctx.enter_context(tc.tile_pool(name="small", bufs=1))
        scrpool = ctx.enter_context(tc.tile_pool(name="scr", bufs=1))
        ypsum = ctx.enter_context(tc.tile_pool(name="yps", bufs=2, space="PSUM"))

        # ---- weight vectors + constants on the gpsimd queue
        wa_bc = cpool.tile([P, C], bf16)
        nc.gpsimd.dma_start(wa_bc[:], wa_bc_d.ap())
        hs = cpool.tile([P, 1], f32)
        nc.gpsimd.dma_start(hs[:], hs_d.ap())
        w3_sb = cpool.tile([P, P], f32)
        nc.gpsimd.dma_start(w3_sb[:], w3_d.ap())
        cars_sb = cpool.tile([P, P], f32)
        nc.gpsimd.dma_start(cars_sb[:], cars_d.ap())
        wp_bc = cpool.tile([P, C], bf16)
        nc.gpsimd.dma_start(wp_bc[:], wp_bc_d.ap())

        # ---- x chunks stream on the SP HWDGE queue
        bases = np.cumsum([0] + CHS[:-1]) * P
        xcs = []
        for c, ch in enumerate(CHS):
            b0 = int(bases[c])
            xc = xpool.tile([P, CHMAX * C], bf16, tag="xc")
            nc.sync.dma_start(
                xc[:, 0:ch * C].rearrange("p (h c) -> p h c", h=ch),
                x_shard.ap()[b0:b0 + ch * P, :]
                .rearrange("(h p) c -> p h c", p=P))
            xcs.append(xc)

        # ---- per-column engine assignment (by global column index j)
        col0 = np.cumsum([0] + CHS[:-1])
        REDUCE_DVE = {3, 7, 11, 15}               # fused stt on DVE
        OUTER_DVE = {1, 3, 5, 7, 9}
        OUTER_ACT = {13, 15}
        # remaining (even) columns' outer on gpsimd

        # ---- main loop, 1-chunk software-pipeline skew
        s_tiles = [None] * NCH

        def emit_s_stage(c, ch):
            xc = xcs[c]
            s_sb = spool.tile([P, CHMAX], f32, tag="s", bufs=3)
            for h in range(ch):
                j = int(col0[c]) + h
                scr = scrpool.tile([P, C], bf16, tag="scr", bufs=2)
                if j in REDUCE_DVE:
                    nc.vector.scalar_tensor_tensor(
                        out=scr[:], in0=xc[:, h * C:(h + 1) * C], scalar=1.0,
                        in1=wa_bc[:], op0=ALU.bypass, op1=ALU.mult,
                        accum_out=s_sb[:, h:h + 1])
                else:
                    nc.vector.tensor_mul(
                        scr[:], xc[:, h * C:(h + 1) * C], wa_bc[:])
                    scrb = scrpool.tile([P, C], bf16, tag="scrb", bufs=2)
                    nc.scalar.activation(scrb[:], scr[:], AF.Copy,
                                         accum_out=s_sb[:, h:h + 1])
            s_tiles[c] = s_sb

        def emit_out_stage(c, ch):
            b0 = int(bases[c])
            s_sb = s_tiles[c]
            # y = W3 @ s (same-column taps) + CARS @ s_shift (carries)
            if c == 0:
                prev_col = hs[:, 0:1]
            else:
                pch = CHS[c - 1]
                prev_col = s_tiles[c - 1][:, pch - 1:pch]
            s_shift = spool.tile([P, CHMAX], f32, tag="sh", bufs=2)
            nc.vector.tensor_scalar_mul(s_shift[:, 0:1], prev_col, 1.0)
            if ch > 1:
                nc.vector.tensor_scalar_mul(
                    s_shift[:, 1:ch], s_sb[:, 0:ch - 1], 1.0)
            y_ps = ypsum.tile([P, CHMAX], f32, tag="yps")
            nc.tensor.matmul(y_ps[:, 0:ch], lhsT=w3_sb[:], rhs=s_sb[:, 0:ch],
                             start=True, stop=False)
            nc.tensor.matmul(y_ps[:, 0:ch], lhsT=cars_sb[:],
                             rhs=s_shift[:, 0:ch], start=False, stop=True)
            y = spool.tile([P, CHMAX], f32, tag="y", bufs=2)
            nc.vector.tensor_scalar_mul(y[:, 0:ch], y_ps[:, 0:ch], 1.0)
            # outer product out[base+128h+p, :] = y[p, h] * wp
            oc = opool.tile([P, CHMAX * C], bf16, tag="oc")
            for h in range(ch):
                j = int(col0[c]) + h
                dst = oc[:, h * C:(h + 1) * C]
                if j in OUTER_DVE:
                    nc.vector.tensor_scalar_mul(dst, wp_bc[:], y[:, h:h + 1])
                elif j in OUTER_ACT:
                    nc.scalar.activation(dst, wp_bc[:], AF.Copy,
                                         scale=y[:, h:h + 1])
                else:
                    nc.gpsimd.tensor_scalar_mul(dst, wp_bc[:], y[:, h:h + 1])
            nc.scalar.dma_start(
                out.ap()[b0:b0 + ch * P, :]
                .rearrange("(h p) c -> p h c", p=P),
                oc[:, 0:ch * C].rearrange("p (h c) -> p h c", h=ch))

        for c in range(NCH + 1):
            if c < NCH:
                emit_s_stage(c, CHS[c])
            if c >= 1:
                emit_out_stage(c - 1, CHS[c - 1])

    nc.compile()
    return nc


def _get_nc():
    if "nc" not in _BUILT:
        _BUILT["nc"] = _build_nc()
    return _BUILT["nc"]


def make_in_maps(x, W_attn, W_proj):
    import ml_dtypes

    bf16 = ml_dtypes.bfloat16
    x = np.asarray(x, dtype=np.float32)
    W_attn = np.asarray(W_attn, dtype=np.float32)
    W_proj = np.asarray(W_proj, dtype=np.float32)

    wa3 = W_attn.sum(axis=0) * (1.0 / 3.0)          # [C], includes the 1/3
    wp = W_proj.sum(axis=1)                          # [C]
    wa_bc = np.ascontiguousarray(np.broadcast_to(wa3.astype(bf16), (P, C)))
    wp_bc = np.ascontiguousarray(np.broadcast_to(wp.astype(bf16), (P, C)))
    w3, cars = _band_consts()
    consts = {"wa_bc": wa_bc, "wp_bc": wp_bc, "w3": w3, "cars": cars}

    x_bf = x.astype(bf16)
    in_maps = []
    for k in range(N_CORES):
        b, h = divmod(k, 2)
        t0 = h * T_LOC
        hs = np.zeros((P, 1), np.float32)
        if h != 0:
            hs[P - 1, 0] = float(x[b, t0 - 1, :] @ wa3)   # s[-1]
            hs[P - 2, 0] = float(x[b, t0 - 2, :] @ wa3)   # s[-2]
        in_maps.append({
            "x_shard": np.ascontiguousarray(x_bf[b, t0:t0 + T_LOC, :]),
            "hs": hs,
            **consts,
        })
    return in_maps


def assemble(results):
    out_full = np.empty((B, T, C), np.float32)
    for k in range(N_CORES):
        b, h = divmod(k, 2)
        t0 = h * T_LOC
        out_full[b, t0:t0 + T_LOC, :] = results[k]["out"].astype(np.float32)
    return out_full


def kernel(x, W_attn, W_proj):
    from concourse.bass_utils import run_bass_kernel_spmd

    nc = _get_nc()
    in_maps = make_in_maps(x, W_attn, W_proj)
    res = run_bass_kernel_spmd(nc, in_maps, list(range(N_CORES)))
    return assemble(res.results)


# revision 17
# speedup vs baseline: 4.1241x; 4.1241x over previous
"""Trainium2 Bass kernel for nn_ConvolutionalSelfAttention.

Mathematical simplification of the reference:
    v[b,t,o]  = sum_c x[b,t,c] W_attn[o,c]
    s[b,t]    = sum_o v[b,t,o] = sum_c x[b,t,c] * wa[c],   wa = colsum(W_attn)
    y[b,t]    = (s[b,t] + s[b,t-1] + s[b,t-2]) / 3        (zero-padded, causal)
    out[b,t,o]= y[b,t] * wp[o],                            wp = rowsum(W_proj)

Sharding (8 cores): core k owns 2048 consecutive tokens of batch k//2
(half k%2).  The tiny reduced weight vectors wa/3 and wp are computed on
the host during input prep (like the halo) and broadcast as needed;
there is no device-side collective.

Schedule (v10):
  - x pre-cast to bf16 on the host; out written as bf16 and upcast in
    assemble().  Device HBM traffic per core: 8 MB in + 8 MB out + ~1 MB
    weights (~46 us roofline at 358 GB/s per core).
  - Token layout: chunk c covers 128*CH_c tokens starting at base_c;
    token t = base_c + 128h + p sits at partition p, column h.  Chunk
    sizes ramp [2,3,3,4,4] so the pipeline fills early.
  - Engine split, balanced against the measured ~67 us copy-only floor
    (v8/v10 were DVE-bound at ~50 us busy).  Per token column: a product
    pass (x*wa), a free-axis reduce, and an outer-product pass
    (y[t]*wp).  These are spread so no engine exceeds ~32 us:
      DVE:    all product passes (bf16 2x); reduce fused via
              scalar_tensor_tensor accum_out for 8/16 columns; outer for
              5/16 columns; tiny copies.  (~39 us)
      ACT:    reduce via activation-Copy accum_out for 8/16 columns;
              outer for 11/16; output DMA issue.  (~40 us)
      GPSIMD: weight/const DMA descriptors only (its tensor ops are
              ~18x slower than DVE and contend for SBUF ports).
      PE:     y via 2 matmuls/chunk: banded stationary W3 (k in
              {m-2,m-1,m}) for same-column taps + carry stationary CARS
              against the shifted column tile (prev column / prev chunk /
              host-prepared halo column hs).
  - Emission is software-pipelined with a 1-chunk skew (s-stage of chunk
    c emitted before y/outer stage of chunk c-1) so ACT's in-order queue
    always has next-chunk reduces available while y_c is in flight.
  - Queues: SP HWDGE = x chunks; gpsimd = weight vectors/constants;
    ACT HWDGE = output chunks.  All DMA payloads bf16 except tiny consts.
  - Rejected alternatives (measured): tensor_tensor_reduce crashes on HW
    (passes CoreSim); gpsimd scalar_tensor_tensor is rejected by
    neuronxcc; DMA-transpose + PE-matvec for s shatters into 256B
    descriptors and is ~2x slower end-to-end (v9: 157 us).
"""

import numpy as np
from contextlib import ExitStack

B, T, C = 4, 4096, 2048
N_CORES = 8
T_LOC = (B * T) // N_CORES      # 2048 tokens per core
P = 128
HC = C // 2
CHS = [2, 3, 3, 4, 4]           # token columns per chunk (x128 tokens)
NCH = len(CHS)
CHMAX = max(CHS)
assert sum(CHS) * P == T_LOC

_BUILT = {}


def _band_consts():
    # lhsT convention: out[m] = sum_k M[k, m] * v[k]
    w3 = np.zeros((P, P), np.float32)
    for m in range(P):
        for k in range(max(0, m - 2), m + 1):
            w3[k, m] = 1.0
    cars = np.zeros((P, P), np.float32)
    cars[P - 1, 0] = 1.0   # t-1 for partition 0
    cars[P - 2, 0] = 1.0   # t-2 for partition 0
    cars[P - 1, 1] = 1.0   # t-2 for partition 1
    return w3, cars


def _build_nc():
    import concourse.tile as tile
    from concourse import bacc, mybir

    f32 = mybir.dt.float32
    bf16 = mybir.dt.bfloat16
    AF = mybir.ActivationFunctionType
    ALU = mybir.AluOpType

    nc = bacc.Bacc("TRN2", target_bir_lowering=False, debug=False,
                   num_devices=N_CORES)

    x_shard = nc.dram_tensor("x_shard", [T_LOC, C], bf16, kind="ExternalInput")
    wa_bc_d = nc.dram_tensor("wa_bc", [P, C], bf16, kind="ExternalInput")
    wp_bc_d = nc.dram_tensor("wp_bc", [P, C], bf16, kind="ExternalInput")
    hs_d = nc.dram_tensor("hs", [P, 1], f32, kind="ExternalInput")
    w3_d = nc.dram_tensor("w3", [P, P], f32, kind="ExternalInput")
    cars_d = nc.dram_tensor("cars", [P, P], f32, kind="ExternalInput")
    out = nc.dram_tensor("out", [T_LOC, C], bf16, kind="ExternalOutput")

    with tile.TileContext(nc) as tc, ExitStack() as ctx:
        cpool = ctx.enter_context(tc.tile_pool(name="const", bufs=1))
        xpool = ctx.enter_context(tc.tile_pool(name="x", bufs=NCH))
        opool = ctx.enter_context(tc.tile_pool(name="o", bufs=2))
        spool = ctx.enter_context(tc.tile_pool(name="small", bufs=1))
        scrpool = ctx.enter_context(tc.tile_pool(name="scr", bufs=1))
        ypsum = ctx.enter_context(tc.tile_pool(name="yps", bufs=2, space="PSUM"))

        # ---- weight vectors + constants on the gpsimd queue
        wa_bc = cpool.tile([P, C], bf16)
        nc.gpsimd.dma_start(wa_bc[:], wa_bc_d.ap())
        hs = cpool.tile([P, 1], f32)
        nc.gpsimd.dma_start(hs[:], hs_d.ap())
        w3_sb = cpool.tile([P, P], f32)
        nc.gpsimd.dma_start(w3_sb[:], w3_d.ap())
        cars_sb = cpool.tile([P, P], f32)
        nc.gpsimd.dma_start(cars_sb[:], cars_d.ap())
        wp_bc = cpool.tile([P, C], bf16)
        nc.gpsimd.dma_start(wp_bc[:], wp_bc_d.ap())

        # ---- x chunks stream on the SP HWDGE queue
        bases = np.cumsum([0] + CHS[:-1]) * P
        xcs = []
        for c, ch in enumerate(CHS):
            b0 = int(bases[c])
            xc = xpool.tile([P, CHMAX * C], bf16, tag="xc")
            nc.sync.dma_start(
                xc[:, 0:ch * C].rearrange("p (h c) -> p h c", h=ch),
                x_shard.ap()[b0:b0 + ch * P, :]
                .rearrange("(h p) c -> p h c", p=P))
            xcs.append(xc)

        # ---- per-column engine assignment (by global column index j)
        col0 = np.cumsum([0] + CHS[:-1])
        REDUCE_DVE = {1, 3, 5, 7, 9, 11, 13, 15}  # fused stt on DVE
        OUTER_DVE = {0, 2, 4, 6, 8}               # rest on ACT
        # NOTE: gpsimd tensor ops measured ~31 us per [128,2048] column
        # (and their SBUF traffic stretches DVE slices) — never offload
        # bulk elementwise to gpsimd.

        # ---- main loop, 1-chunk software-pipeline skew
        s_tiles = [None] * NCH

        def emit_s_stage(c, ch):
            xc = xcs[c]
            s_sb = spool.tile([P, CHMAX], f32, tag="s", bufs=3)
            for h in range(ch):
                j = int(col0[c]) + h
                scr = scrpool.tile([P, C], bf16, tag="scr", bufs=2)
                if j in REDUCE_DVE:
                    nc.vector.scalar_tensor_tensor(
                        out=scr[:], in0=xc[:, h * C:(h + 1) * C], scalar=1.0,
                        in1=wa_bc[:], op0=ALU.bypass, op1=ALU.mult,
                        accum_out=s_sb[:, h:h + 1])
                else:
                    nc.vector.tensor_mul(
                        scr[:], xc[:, h * C:(h + 1) * C], wa_bc[:])
                    scrb = scrpool.tile([P, C], bf16, tag="scrb", bufs=2)
                    nc.scalar.activation(scrb[:], scr[:], AF.Copy,
                                         accum_out=s_sb[:, h:h + 1])
            s_tiles[c] = s_sb

        def emit_out_stage(c, ch):
            b0 = int(bases[c])
            s_sb = s_tiles[c]
            # y = W3 @ s (same-column taps) + CARS @ s_shift (carries)
            if c == 0:
                prev_col = hs[:, 0:1]
            else:
                pch = CHS[c - 1]
                prev_col = s_tiles[c - 1][:, pch - 1:pch]
            s_shift = spool.tile([P, CHMAX], f32, tag="sh", bufs=2)
            nc.vector.tensor_scalar_mul(s_shift[:, 0:1], prev_col, 1.0)
            if ch > 1:
                nc.vector.tensor_scalar_mul(
                    s_shift[:, 1:ch], s_sb[:, 0:ch - 1], 1.0)
            y_ps = ypsum.tile([P, CHMAX], f32, tag="yps")
            nc.tensor.matmul(y_ps[:, 0:ch], lhsT=w3_sb[:], rhs=s_sb[:, 0:ch],
                             start=True, stop=False)
            nc.tensor.matmul(y_ps[:, 0:ch], lhsT=cars_sb[:],
                             rhs=s_shift[:, 0:ch], start=False, stop=True)
            y = spool.tile([P, CHMAX], f32, tag="y", bufs=2)
            nc.vector.tensor_scalar_mul(y[:, 0:ch], y_ps[:, 0:ch], 1.0)
            # outer product out[base+128h+p, :] = y[p, h] * wp
            oc = opool.tile([P, CHMAX * C], bf16, tag="oc")
            for h in range(ch):
                j = int(col0[c]) + h
                dst = oc[:, h * C:(h + 1) * C]
                if j in OUTER_DVE:
                    nc.vector.tensor_scalar_mul(dst, wp_bc[:], y[:, h:h + 1])
                else:
                    nc.scalar.activation(dst, wp_bc[:], AF.Copy,
                                         scale=y[:, h:h + 1])
            nc.scalar.dma_start(
                out.ap()[b0:b0 + ch * P, :]
                .rearrange("(h p) c -> p h c", p=P),
                oc[:, 0:ch * C].rearrange("p (h c) -> p h c", h=ch))

        for c in range(NCH + 1):
            if c < NCH:
                emit_s_stage(c, CHS[c])
            if c >= 1:
                emit_out_stage(c - 1, CHS[c - 1])

    nc.compile()
    return nc


def _get_nc():
    if "nc" not in _BUILT:
        _BUILT["nc"] = _build_nc()
    return _BUILT["nc"]


def make_in_maps(x, W_attn, W_proj):
    import ml_dtypes

    bf16 = ml_dtypes.bfloat16
    x = np.asarray(x, dtype=np.float32)
    W_attn = np.asarray(W_attn, dtype=np.float32)
    W_proj = np.asarray(W_proj, dtype=np.float32)

    wa3 = W_attn.sum(axis=0) * (1.0 / 3.0)          # [C], includes the 1/3
    wp = W_proj.sum(axis=1)                          # [C]
    wa_bc = np.ascontiguousarray(np.broadcast_to(wa3.astype(bf16), (P, C)))
    wp_bc = np.ascontiguousarray(np.broadcast_to(wp.astype(bf16), (P, C)))
    w3, cars = _band_consts()
    consts = {"wa_bc": wa_bc, "wp_bc": wp_bc, "w3": w3, "cars": cars}

    x_bf = x.astype(bf16)
    in_maps = []
    for k in range(N_CORES):
        b, h = divmod(k, 2)
        t0 = h * T_LOC
        hs = np.zeros((P, 1), np.float32)
        if h != 0:
            hs[P - 1, 0] = float(x[b, t0 - 1, :] @ wa3)   # s[-1]
            hs[P - 2, 0] = float(x[b, t0 - 2, :] @ wa3)   # s[-2]
        in_maps.append({
            "x_shard": np.ascontiguousarray(x_bf[b, t0:t0 + T_LOC, :]),
            "hs": hs,
            **consts,
        })
    return in_maps


def assemble(results):
    out_full = np.empty((B, T, C), np.float32)
    for k in range(N_CORES):
        b, h = divmod(k, 2)
        t0 = h * T_LOC
        out_full[b, t0:t0 + T_LOC, :] = results[k]["out"].astype(np.float32)
    return out_full


def kernel(x, W_attn, W_proj):
    from concourse.bass_utils import run_bass_kernel_spmd

    nc = _get_nc()
    in_maps = make_in_maps(x, W_attn, W_proj)
    res = run_bass_kernel_spmd(nc, in_maps, list(range(N_CORES)))
    return assemble(res.results)


# revision 18
# speedup vs baseline: 4.3032x; 1.0434x over previous
"""Trainium2 Bass kernel for nn_ConvolutionalSelfAttention.

Mathematical simplification of the reference:
    v[b,t,o]  = sum_c x[b,t,c] W_attn[o,c]
    s[b,t]    = sum_o v[b,t,o] = sum_c x[b,t,c] * wa[c],   wa = colsum(W_attn)
    y[b,t]    = (s[b,t] + s[b,t-1] + s[b,t-2]) / 3        (zero-padded, causal)
    out[b,t,o]= y[b,t] * wp[o],                            wp = rowsum(W_proj)

Sharding (8 cores): core k owns 2048 consecutive tokens of batch k//2
(half k%2).  The tiny reduced weight vectors wa/3 and wp are computed on
the host during input prep (like the halo) and broadcast as needed;
there is no device-side collective.

Schedule (v10):
  - x pre-cast to bf16 on the host; out written as bf16 and upcast in
    assemble().  Device HBM traffic per core: 8 MB in + 8 MB out + ~1 MB
    weights (~46 us roofline at 358 GB/s per core).
  - Token layout: chunk c covers 128*CH_c tokens starting at base_c;
    token t = base_c + 128h + p sits at partition p, column h.  Chunk
    sizes ramp [2,3,3,4,4] so the pipeline fills early.
  - Engine split, balanced against the measured ~67 us copy-only floor
    (v8/v10 were DVE-bound at ~50 us busy).  Per token column: a product
    pass (x*wa), a free-axis reduce, and an outer-product pass
    (y[t]*wp).  These are spread so no engine exceeds ~32 us:
      DVE:    all product passes (bf16 2x); reduce fused via
              scalar_tensor_tensor accum_out for 8/16 columns; outer for
              5/16 columns; tiny copies.  (~39 us)
      ACT:    reduce via activation-Copy accum_out for 8/16 columns;
              outer for 11/16; output DMA issue.  (~40 us)
      GPSIMD: weight/const DMA descriptors only (its tensor ops are
              ~18x slower than DVE and contend for SBUF ports).
      PE:     y via 2 matmuls/chunk: banded stationary W3 (k in
              {m-2,m-1,m}) for same-column taps + carry stationary CARS
              against the shifted column tile (prev column / prev chunk /
              host-prepared halo column hs).
  - Emission is software-pipelined with a 1-chunk skew (s-stage of chunk
    c emitted before y/outer stage of chunk c-1) so ACT's in-order queue
    always has next-chunk reduces available while y_c is in flight.
  - Queues: SP HWDGE = x chunks; gpsimd = weight vectors/constants;
    ACT HWDGE = output chunks.  All DMA payloads bf16 except tiny consts.
  - Rejected alternatives (measured): tensor_tensor_reduce crashes on HW
    (passes CoreSim); gpsimd scalar_tensor_tensor is rejected by
    neuronxcc; DMA-transpose + PE-matvec for s shatters into 256B
    descriptors and is ~2x slower end-to-end (v9: 157 us).
"""

import numpy as np
from contextlib import ExitStack

B, T, C = 4, 4096, 2048
N_CORES = 8
T_LOC = (B * T) // N_CORES      # 2048 tokens per core
P = 128
HC = C // 2
CHS = [2, 3, 3, 4, 4]           # token columns per chunk (x128 tokens)
NCH = len(CHS)
CHMAX = max(CHS)
assert sum(CHS) * P == T_LOC

_BUILT = {}


def _band_consts():
    # lhsT convention: out[m] = sum_k M[k, m] * v[k]
    w3 = np.zeros((P, P), np.float32)
    for m in range(P):
        for k in range(max(0, m - 2), m + 1):
            w3[k, m] = 1.0
    cars = np.zeros((P, P), np.float32)
    cars[P - 1, 0] = 1.0   # t-1 for partition 0
    cars[P - 2, 0] = 1.0   # t-2 for partition 0
    cars[P - 1, 1] = 1.0   # t-2 for partition 1
    return w3, cars


def _build_nc():
    import concourse.tile as tile
    from concourse import bacc, mybir

    f32 = mybir.dt.float32
    bf16 = mybir.dt.bfloat16
    AF = mybir.ActivationFunctionType
    ALU = mybir.AluOpType

    nc = bacc.Bacc("TRN2", target_bir_lowering=False, debug=False,
                   num_devices=N_CORES)

    x_shard = nc.dram_tensor("x_shard", [T_LOC, C], bf16, kind="ExternalInput")
    wa_bc_d = nc.dram_tensor("wa_bc", [P, C], bf16, kind="ExternalInput")
    wp_bc_d = nc.dram_tensor("wp_bc", [P, C], bf16, kind="ExternalInput")
    hs_d = nc.dram_tensor("hs", [P, 1], f32, kind="ExternalInput")
    w3_d = nc.dram_tensor("w3", [P, P], f32, kind="ExternalInput")
    cars_d = nc.dram_tensor("cars", [P, P], f32, kind="ExternalInput")
    out = nc.dram_tensor("out", [T_LOC, C], bf16, kind="ExternalOutput")

    with tile.TileContext(nc) as tc, ExitStack() as ctx:
        cpool = ctx.enter_context(tc.tile_pool(name="const", bufs=1))
        xpool = ctx.enter_context(tc.tile_pool(name="x", bufs=NCH))
        opool = ctx.enter_context(tc.tile_pool(name="o", bufs=2))
        spool = ctx.enter_context(tc.tile_pool(name="small", bufs=1))
        scrpool = ctx.enter_context(tc.tile_pool(name="scr", bufs=1))
        ypsum = ctx.enter_context(tc.tile_pool(name="yps", bufs=2, space="PSUM"))

        # ---- weight vectors + constants on the gpsimd queue
        wa_bc = cpool.tile([P, C], bf16)
        nc.gpsimd.dma_start(wa_bc[:], wa_bc_d.ap())
        hs = cpool.tile([P, 1], f32)
        nc.gpsimd.dma_start(hs[:], hs_d.ap())
        w3_sb = cpool.tile([P, P], f32)
        nc.gpsimd.dma_start(w3_sb[:], w3_d.ap())
        cars_sb = cpool.tile([P, P], f32)
        nc.gpsimd.dma_start(cars_sb[:], cars_d.ap())
        wp_bc = cpool.tile([P, C], bf16)
        nc.gpsimd.dma_start(wp_bc[:], wp_bc_d.ap())

        # ---- x chunks stream on the SP HWDGE queue
        bases = np.cumsum([0] + CHS[:-1]) * P
        xcs = []
        for c, ch in enumerate(CHS):
            b0 = int(bases[c])
            xc = xpool.tile([P, CHMAX * C], bf16, tag="xc")
            nc.sync.dma_start(
                xc[:, 0:ch * C].rearrange("p (h c) -> p h c", h=ch),
                x_shard.ap()[b0:b0 + ch * P, :]
                .rearrange("(h p) c -> p h c", p=P))
            xcs.append(xc)

        # ---- per-column engine assignment (by global column index j)
        col0 = np.cumsum([0] + CHS[:-1])
        REDUCE_DVE = {1, 3, 5, 7, 9, 11, 13, 15}  # fused stt on DVE
        OUTER_DVE = {0, 2, 4, 6, 8, 10, 12, 14}   # rest on ACT
        # NOTE: gpsimd tensor ops measured ~31 us per [128,2048] column
        # (and their SBUF traffic stretches DVE slices) — never offload
        # bulk elementwise to gpsimd.

        # ---- main loop, 1-chunk software-pipeline skew
        s_tiles = [None] * NCH

        def emit_s_stage(c, ch):
            xc = xcs[c]
            s_sb = spool.tile([P, CHMAX], f32, tag="s", bufs=3)
            for h in range(ch):
                j = int(col0[c]) + h
                scr = scrpool.tile([P, C], bf16, tag="scr", bufs=2)
                if j in REDUCE_DVE:
                    nc.vector.scalar_tensor_tensor(
                        out=scr[:], in0=xc[:, h * C:(h + 1) * C], scalar=1.0,
                        in1=wa_bc[:], op0=ALU.bypass, op1=ALU.mult,
                        accum_out=s_sb[:, h:h + 1])
                else:
                    nc.vector.tensor_mul(
                        scr[:], xc[:, h * C:(h + 1) * C], wa_bc[:])
                    scrb = scrpool.tile([P, C], bf16, tag="scrb", bufs=2)
                    nc.scalar.activation(scrb[:], scr[:], AF.Copy,
                                         accum_out=s_sb[:, h:h + 1])
            s_tiles[c] = s_sb

        def emit_out_stage(c, ch):
            b0 = int(bases[c])
            s_sb = s_tiles[c]
            # y = W3 @ s (same-column taps) + CARS @ s_shift (carries)
            if c == 0:
                prev_col = hs[:, 0:1]
            else:
                pch = CHS[c - 1]
                prev_col = s_tiles[c - 1][:, pch - 1:pch]
            s_shift = spool.tile([P, CHMAX], f32, tag="sh", bufs=2)
            nc.vector.tensor_scalar_mul(s_shift[:, 0:1], prev_col, 1.0)
            if ch > 1:
                nc.vector.tensor_scalar_mul(
                    s_shift[:, 1:ch], s_sb[:, 0:ch - 1], 1.0)
            y_ps = ypsum.tile([P, CHMAX], f32, tag="yps")
            nc.tensor.matmul(y_ps[:, 0:ch], lhsT=w3_sb[:], rhs=s_sb[:, 0:ch],
                             start=True, stop=False)
            nc.tensor.matmul(y_ps[:, 0:ch], lhsT=cars_sb[:],
                             rhs=s_shift[:, 0:ch], start=False, stop=True)
            y = spool.tile([P, CHMAX], f32, tag="y", bufs=2)
            nc.vector.tensor_scalar_mul(y[:, 0:ch], y_ps[:, 0:ch], 1.0)
            # outer product out[base+128h+p, :] = y[p, h] * wp
            oc = opool.tile([P, CHMAX * C], bf16, tag="oc")
            for h in range(ch):
                j = int(col0[c]) + h
                dst = oc[:, h * C:(h + 1) * C]
                if j in OUTER_DVE:
                    nc.vector.tensor_scalar_mul(dst, wp_bc[:], y[:, h:h + 1])
                else:
                    nc.scalar.activation(dst, wp_bc[:], AF.Copy,
                                         scale=y[:, h:h + 1])
            nc.scalar.dma_start(
                out.ap()[b0:b0 + ch * P, :]
                .rearrange("(h p) c -> p h c", p=P),
                oc[:, 0:ch * C].rearrange("p (h c) -> p h c", h=ch))

        for c in range(NCH + 1):
            if c < NCH:
                emit_s_stage(c, CHS[c])
            if c >= 1:
                emit_out_stage(c - 1, CHS[c - 1])

    nc.compile()
    return nc


def _get_nc():
    if "nc" not in _BUILT:
        _BUILT["nc"] = _build_nc()
    return _BUILT["nc"]


def make_in_maps(x, W_attn, W_proj):
    import ml_dtypes

    bf16 = ml_dtypes.bfloat16
    x = np.asarray(x, dtype=np.float32)
    W_attn = np.asarray(W_attn, dtype=np.float32)
    W_proj = np.asarray(W_proj, dtype=np.float32)

    wa3 = W_attn.sum(axis=0) * (1.0 / 3.0)          # [C], includes the 1/3
    wp = W_proj.sum(axis=1)                          # [C]
    wa_bc = np.ascontiguousarray(np.broadcast_to(wa3.astype(bf16), (P, C)))
    wp_bc = np.ascontiguousarray(np.broadcast_to(wp.astype(bf16), (P, C)))
    w3, cars = _band_consts()
    consts = {"wa_bc": wa_bc, "wp_bc": wp_bc, "w3": w3, "cars": cars}

    x_bf = x.astype(bf16)
    in_maps = []
    for k in range(N_CORES):
        b, h = divmod(k, 2)
        t0 = h * T_LOC
        hs = np.zeros((P, 1), np.float32)
        if h != 0:
            hs[P - 1, 0] = float(x[b, t0 - 1, :] @ wa3)   # s[-1]
            hs[P - 2, 0] = float(x[b, t0 - 2, :] @ wa3)   # s[-2]
        in_maps.append({
            "x_shard": np.ascontiguousarray(x_bf[b, t0:t0 + T_LOC, :]),
            "hs": hs,
            **consts,
        })
    return in_maps


def assemble(results):
    out_full = np.empty((B, T, C), np.float32)
    for k in range(N_CORES):
        b, h = divmod(k, 2)
        t0 = h * T_LOC
        out_full[b, t0:t0 + T_LOC, :] = results[k]["out"].astype(np.float32)
    return out_full


def kernel(x, W_attn, W_proj):
    from concourse.bass_utils import run_bass_kernel_spmd

    nc = _get_nc()
    in_maps = make_in_maps(x, W_attn, W_proj)
    res = run_bass_kernel_spmd(nc, in_maps, list(range(N_CORES)))
    return assemble(res.results)
